# revision 5
# baseline (speedup 1.0000x reference)
"""Trainium2 Bass kernel for nn_NeuralNetGlobalHammer.

Math (per element, on the complex signal z = xr + i*xi):
    t    = xr^2 + xi^2
    mag  = sqrt(t)
    s    = sum_j w2[j] * tanh(w1[j] * mag)        (8-channel MLP)
    p    = s / mag
    xr'  = p * xr ;  xi' = p * xi                 (re-attach phase)
    yr   = conv(xr', wr) - conv(xi', wi)          (32-tap valid FIR along W)
    yi   = conv(xi', wr) + conv(xr', wi)
    out  = SCALE * stack([yr, yi], -1)

Mapping: pure data parallel, core c gets rows [128c, 128c+128) of the
(B*H = 1024, W = 16384) row-major view.  Pointwise runs row-major on
ACT/DVE; the FIR runs on the PE as fp32 matmuls with the transposed
data chunk as the stationary operand and a banded Toeplitz filter
matrix as the moving operand, so conv outputs land row-major in PSUM.
"""
import numpy as np

B, H, W = 16, 64, 16384
FL = 32                      # filter taps
WOUT = W - FL + 1            # 16353
ROWS = 128                   # rows per core (B*H / 8)
NCORES = 8
STRIDE = 96                  # FIR outputs per PE window (128 - 32)
CW = 16 * STRIDE             # 1536 output cols per chunk
X = CW + FL                  # 1568 input cols per chunk tile
EXPECTED_SI_POWER_DB = -15
SCALE = float(np.sqrt(10.0 ** (EXPECTED_SI_POWER_DB / 10.0)))

_CACHE = {}


def _build_program(w1, w2):
    import concourse.bass as bass
    import concourse.bacc as bacc
    import concourse.mybir as mybir
    import concourse.tile as tile
    from concourse.dve_ops import RECIPROCAL_APPROX_FAST, RECIP_APPROX_FAST_CONSTS

    F32 = mybir.dt.float32
    AF = mybir.ActivationFunctionType
    OP = mybir.AluOpType
    RC = RECIP_APPROX_FAST_CONSTS

    nc = bacc.Bacc("TRN2", target_bir_lowering=False, debug=False,
                   num_devices=NCORES)

    xr_d = nc.declare_dram_parameter("xr", [ROWS, W], F32, isOutput=False)
    xi_d = nc.declare_dram_parameter("xi", [ROWS, W], F32, isOutput=False)
    tr_d = nc.declare_dram_parameter("toe_r", [128, STRIDE], F32, isOutput=False)
    ti_d = nc.declare_dram_parameter("toe_i", [128, STRIDE], F32, isOutput=False)
    tni_d = nc.declare_dram_parameter("toe_ni", [128, STRIDE], F32, isOutput=False)
    eye_d = nc.declare_dram_parameter("eye", [128, 128], F32, isOutput=False)
    out_d = nc.declare_dram_parameter("out", [ROWS, 2 * WOUT], F32, isOutput=True)

    # chunk layout along W
    bases = list(range(0, W, CW))                      # 11 bases, last = 15360
    chunks = []
    for cb in bases:
        lw = min(X, W - cb)                            # valid input width
        nwin = min(16, (WOUT - cb + STRIDE - 1) // STRIDE)
        vout = min(CW, WOUT - cb)                      # valid output cols
        chunks.append((cb, lw, nwin, vout))

    with tile.TileContext(nc) as tc:
        with (
            tc.tile_pool(name="const", bufs=1) as cpool,
            tc.tile_pool(name="io", bufs=2) as iop,
            tc.tile_pool(name="mid", bufs=2) as midp,
            tc.tile_pool(name="hb", bufs=3) as hbp,
            tc.tile_pool(name="xtb", bufs=3) as xtbp,
            tc.tile_pool(name="stg", bufs=2) as stgp,
            tc.tile_pool(name="tps", bufs=3, space=bass.MemorySpace.PSUM) as tpsp,
            tc.tile_pool(name="ypr", bufs=2, space=bass.MemorySpace.PSUM) as yprp,
            tc.tile_pool(name="ypi", bufs=2, space=bass.MemorySpace.PSUM) as ypip,
        ):
            tr_t = cpool.tile([128, STRIDE], F32, tag="tr")
            ti_t = cpool.tile([128, STRIDE], F32, tag="ti")
            tni_t = cpool.tile([128, STRIDE], F32, tag="tni")
            eye_t = cpool.tile([128, 128], F32, tag="eye")
            nc.sync.dma_start(tr_t[:], tr_d[:])
            nc.sync.dma_start(ti_t[:], ti_d[:])
            nc.sync.dma_start(tni_t[:], tni_d[:])
            nc.sync.dma_start(eye_t[:], eye_d[:])

            for (cb, lw, nwin, vout) in chunks:
                xr_t = iop.tile([ROWS, X], F32, tag="xr")
                xi_t = iop.tile([ROWS, X], F32, tag="xi")
                if lw < X:
                    nc.vector.memset(xr_t[:, lw:X], 0.0)
                    nc.vector.memset(xi_t[:, lw:X], 0.0)
                nc.sync.dma_start(xr_t[:, 0:lw], xr_d[:, cb:cb + lw])
                nc.sync.dma_start(xi_t[:, 0:lw], xi_d[:, cb:cb + lw])

                sq_r = midp.tile([ROWS, X], F32, tag="sq_r")
                nc.scalar.activation(sq_r[:], xr_t[:], AF.Square)
                sq_i = midp.tile([ROWS, X], F32, tag="sq_i")
                nc.scalar.activation(sq_i[:], xi_t[:], AF.Square)
                t_t = midp.tile([ROWS, X], F32, tag="t")
                nc.vector.tensor_add(t_t[:], sq_r[:], sq_i[:])
                mag = midp.tile([ROWS, X], F32, tag="mag")
                nc.scalar.activation(mag[:], t_t[:], AF.Sqrt)
                inv = midp.tile([ROWS, X], F32, tag="inv")
                nc.vector._custom_dve(RECIPROCAL_APPROX_FAST, out=inv[:],
                                      in0=mag[:], s0=RC["s0"], s1=RC["s1"],
                                      imm2=RC["imm2"])
                s_prev = None
                for j in range(8):
                    h_t = hbp.tile([ROWS, X], F32, tag="h")
                    nc.scalar.activation(h_t[:], mag[:], AF.Tanh, scale=float(w1[j]))
                    s_new = hbp.tile([ROWS, X], F32, tag="s")
                    if s_prev is None:
                        nc.vector.tensor_scalar_mul(s_new[:], h_t[:], float(w2[j]))
                    else:
                        nc.vector.scalar_tensor_tensor(
                            s_new[:], h_t[:], float(w2[j]), s_prev[:],
                            OP.mult, OP.add)
                    s_prev = s_new
                p_t = midp.tile([ROWS, X], F32, tag="p")
                nc.vector.tensor_mul(p_t[:], s_prev[:], inv[:])
                xp_r = midp.tile([ROWS, X], F32, tag="xp_r")
                nc.vector.tensor_mul(xp_r[:], p_t[:], xr_t[:])
                xp_i = midp.tile([ROWS, X], F32, tag="xp_i")
                nc.vector.tensor_mul(xp_i[:], p_t[:], xi_t[:])
                if lw < X:
                    # pad region holds NaN (0/0); zero it so the Toeplitz
                    # band zeros actually mask it in the FIR matmuls
                    nc.vector.memset(xp_r[:, lw:X], 0.0)
                    nc.vector.memset(xp_i[:, lw:X], 0.0)

                stg = stgp.tile([ROWS, 2 * CW], F32, tag="stg")

                # windows, processed in pairs sharing PSUM banks
                for pj in range(0, nwin, 2):
                    npair = min(2, nwin - pj)
                    tb = tpsp.tile([128, 512], F32, tag="tps")
                    for u in range(npair):
                        j = pj + u
                        o = 256 * u
                        nc.tensor.transpose(
                            tb[:, o:o + 128], xp_r[:, STRIDE * j:STRIDE * j + 128],
                            eye_t[:])
                        nc.tensor.transpose(
                            tb[:, o + 128:o + 256],
                            xp_i[:, STRIDE * j:STRIDE * j + 128], eye_t[:])
                    xtb = xtbp.tile([128, 512], F32, tag="xtb")
                    if (pj // 2) % 2 == 0:
                        nc.vector.tensor_copy(xtb[:, 0:256 * npair], tb[:, 0:256 * npair])
                    else:
                        nc.scalar.copy(xtb[:, 0:256 * npair], tb[:, 0:256 * npair])

                    for u in range(npair):
                        j = pj + u
                        o = 256 * u
                        xrT = xtb[:, o:o + 128]
                        xiT = xtb[:, o + 128:o + 256]
                        ypr = yprp.tile([128, STRIDE], F32, tag="ypr")
                        ypi = ypip.tile([128, STRIDE], F32, tag="ypi")
                        nc.tensor.matmul(ypr[:], xrT, tr_t[:],
                                         start=True, stop=False)
                        nc.tensor.matmul(ypi[:], xrT, ti_t[:],
                                         start=True, stop=False)
                        nc.tensor.matmul(ypr[:], xiT, tni_t[:],
                                         start=False, stop=True)
                        nc.tensor.matmul(ypi[:], xiT, tr_t[:],
                                         start=False, stop=True)
                        # evacuate with scale + re/im interleave
                        so = 192 * j
                        if j % 2 == 0:
                            nc.scalar.mul(stg[:, so:so + 192:2], ypr[:], SCALE)
                            nc.vector.tensor_scalar_mul(
                                stg[:, so + 1:so + 192:2], ypi[:], SCALE)
                        else:
                            nc.vector.tensor_scalar_mul(
                                stg[:, so:so + 192:2], ypr[:], SCALE)
                            nc.scalar.mul(stg[:, so + 1:so + 192:2], ypi[:], SCALE)

                nc.sync.dma_start(out_d[:, 2 * cb:2 * (cb + vout)],
                                  stg[:, 0:2 * vout])

    nc.compile()
    return nc


def _get_program(w1, w2):
    key = (w1.tobytes(), w2.tobytes())
    if key not in _CACHE:
        _CACHE[key] = _build_program(w1, w2)
    return _CACHE[key]


def _toeplitz(taps, sign=1.0):
    t = np.zeros((128, STRIDE), dtype=np.float32)
    for m in range(STRIDE):
        t[m:m + FL, m] = sign * taps
    return t


def kernel(x_real, x_imag, w_nl1, w_nl2, w_lin_real, w_lin_imag,
           _trace=False, _trace_kwargs=None):
    from concourse.bass_utils import run_bass_kernel_spmd

    w1 = np.asarray(w_nl1, dtype=np.float32).reshape(8)
    w2 = np.asarray(w_nl2, dtype=np.float32).reshape(8)
    wr = np.asarray(w_lin_real, dtype=np.float32).reshape(FL)
    wi = np.asarray(w_lin_imag, dtype=np.float32).reshape(FL)

    nc = _get_program(w1, w2)

    xr = np.ascontiguousarray(np.asarray(x_real, np.float32).reshape(B * H, W))
    xi = np.ascontiguousarray(np.asarray(x_imag, np.float32).reshape(B * H, W))
    consts = {
        "toe_r": _toeplitz(wr),
        "toe_i": _toeplitz(wi),
        "toe_ni": _toeplitz(wi, -1.0),
        "eye": np.eye(128, dtype=np.float32),
    }
    in_maps = []
    for c in range(NCORES):
        in_maps.append({
            "xr": np.ascontiguousarray(xr[ROWS * c:ROWS * (c + 1)]),
            "xi": np.ascontiguousarray(xi[ROWS * c:ROWS * (c + 1)]),
            **consts,
        })
    kw = {}
    if _trace:
        kw["trace"] = True
        if _trace_kwargs:
            kw.update(_trace_kwargs)
    res = run_bass_kernel_spmd(nc, in_maps, list(range(NCORES)), **kw)
    out = np.concatenate([res.results[c]["out"].reshape(ROWS, WOUT, 2)
                          for c in range(NCORES)], axis=0)
    out = out.reshape(B, H, WOUT, 1, 2)
    if _trace:
        kernel.last_results = res
    return out


# revision 9
# speedup vs baseline: 261.4709x; 261.4709x over previous
"""Trainium2 Bass kernel for nn_NeuralNetGlobalHammer.

Math (per element, on the complex signal z = xr + i*xi):
    t    = xr^2 + xi^2
    mag  = sqrt(t)
    s    = sum_j w2[j] * tanh(w1[j] * mag)        (8-channel MLP)
    p    = s / mag
    xr'  = p * xr ;  xi' = p * xi                 (re-attach phase)
    yr   = conv(xr', wr) - conv(xi', wi)          (32-tap valid FIR along W)
    yi   = conv(xi', wr) + conv(xr', wi)
    out  = SCALE * stack([yr, yi], -1)

Mapping: pure data parallel, core c gets rows [128c, 128c+128) of the
(B*H = 1024, W = 16384) row-major view.  Pointwise runs row-major on
ACT/DVE; the FIR runs on the PE as fp32 matmuls with the transposed
data chunk as the stationary operand and a banded Toeplitz filter
matrix as the moving operand, so conv outputs land row-major in PSUM.
"""
import numpy as np

B, H, W = 16, 64, 16384
FL = 32                      # filter taps
WOUT = W - FL + 1            # 16353
ROWS = 128                   # rows per core (B*H / 8)
NCORES = 8
STRIDE = 96                  # FIR outputs per PE window (128 - 32)
CW = 16 * STRIDE             # 1536 output cols per chunk
X = CW + FL                  # 1568 input cols per chunk tile
EXPECTED_SI_POWER_DB = -15
SCALE = float(np.sqrt(10.0 ** (EXPECTED_SI_POWER_DB / 10.0)))

_CACHE = {}


def _build_program(w1, w2, n_reps=1):
    import contextlib
    import concourse.bass as bass
    import concourse.bacc as bacc
    import concourse.mybir as mybir
    import concourse.tile as tile
    from concourse.dve_ops import RECIPROCAL_APPROX_FAST, RECIP_APPROX_FAST_CONSTS

    F32 = mybir.dt.float32
    AF = mybir.ActivationFunctionType
    OP = mybir.AluOpType
    RC = RECIP_APPROX_FAST_CONSTS

    nc = bacc.Bacc("TRN2", target_bir_lowering=False, debug=False,
                   num_devices=NCORES)

    xr_d = nc.declare_dram_parameter("xr", [ROWS, W], F32, isOutput=False)
    xi_d = nc.declare_dram_parameter("xi", [ROWS, W], F32, isOutput=False)
    tr_d = nc.declare_dram_parameter("toe_r", [128, STRIDE], F32, isOutput=False)
    ti_d = nc.declare_dram_parameter("toe_i", [128, STRIDE], F32, isOutput=False)
    tni_d = nc.declare_dram_parameter("toe_ni", [128, STRIDE], F32, isOutput=False)
    eye_d = nc.declare_dram_parameter("eye", [128, 128], F32, isOutput=False)
    out_d = nc.declare_dram_parameter("out", [ROWS, 2 * WOUT], F32, isOutput=True)

    # chunk layout along W
    bases = list(range(0, W, CW))                      # 11 bases, last = 15360
    chunks = []
    for cb in bases:
        lw = min(X, W - cb)                            # valid input width
        nwin = min(16, (WOUT - cb + STRIDE - 1) // STRIDE)
        vout = min(CW, WOUT - cb)                      # valid output cols
        chunks.append((cb, lw, nwin, vout))

    with tile.TileContext(nc) as tc:
        with (
            tc.tile_pool(name="const", bufs=1) as cpool,
            tc.tile_pool(name="io", bufs=2) as iop,
            tc.tile_pool(name="mid", bufs=2) as midp,
            tc.tile_pool(name="hb", bufs=3) as hbp,
            tc.tile_pool(name="xtb", bufs=3) as xtbp,
            tc.tile_pool(name="stg", bufs=2) as stgp,
            tc.tile_pool(name="tps", bufs=3, space=bass.MemorySpace.PSUM) as tpsp,
            tc.tile_pool(name="ypr", bufs=2, space=bass.MemorySpace.PSUM) as yprp,
            tc.tile_pool(name="ypi", bufs=2, space=bass.MemorySpace.PSUM) as ypip,
        ):
            tr_t = cpool.tile([128, STRIDE], F32, tag="tr")
            ti_t = cpool.tile([128, STRIDE], F32, tag="ti")
            tni_t = cpool.tile([128, STRIDE], F32, tag="tni")
            eye_t = cpool.tile([128, 128], F32, tag="eye")
            nc.sync.dma_start(tr_t[:], tr_d[:])
            nc.sync.dma_start(ti_t[:], ti_d[:])
            nc.sync.dma_start(tni_t[:], tni_d[:])
            nc.sync.dma_start(eye_t[:], eye_d[:])

            rep_ctx = (tc.For_i(0, n_reps, 1) if n_reps > 1
                       else contextlib.nullcontext())
            with rep_ctx:
                _emit_body(nc, tc, chunks, w1, w2,
                           iop, midp, hbp, xtbp, stgp, tpsp, yprp, ypip,
                           xr_d, xi_d, out_d, tr_t, ti_t, tni_t, eye_t,
                           F32, AF, OP, RECIPROCAL_APPROX_FAST, RC)

    nc.compile()
    return nc


def _emit_body(nc, tc, chunks, w1, w2,
               iop, midp, hbp, xtbp, stgp, tpsp, yprp, ypip,
               xr_d, xi_d, out_d, tr_t, ti_t, tni_t, eye_t,
               F32, AF, OP, RECIPROCAL_APPROX_FAST, RC):
    if True:
            for (cb, lw, nwin, vout) in chunks:
                xr_t = iop.tile([ROWS, X], F32, tag="xr")
                xi_t = iop.tile([ROWS, X], F32, tag="xi")
                if lw < X:
                    nc.vector.memset(xr_t[:, lw:X], 0.0)
                    nc.vector.memset(xi_t[:, lw:X], 0.0)
                nc.sync.dma_start(xr_t[:, 0:lw], xr_d[:, cb:cb + lw])
                nc.sync.dma_start(xi_t[:, 0:lw], xi_d[:, cb:cb + lw])

                sq_r = midp.tile([ROWS, X], F32, tag="sq_r")
                nc.scalar.activation(sq_r[:], xr_t[:], AF.Square)
                sq_i = midp.tile([ROWS, X], F32, tag="sq_i")
                nc.scalar.activation(sq_i[:], xi_t[:], AF.Square)
                t_t = midp.tile([ROWS, X], F32, tag="t")
                nc.vector.tensor_add(t_t[:], sq_r[:], sq_i[:])
                mag = midp.tile([ROWS, X], F32, tag="mag")
                nc.scalar.activation(mag[:], t_t[:], AF.Sqrt)
                inv = midp.tile([ROWS, X], F32, tag="inv")
                nc.vector._custom_dve(RECIPROCAL_APPROX_FAST, out=inv[:],
                                      in0=mag[:], s0=RC["s0"], s1=RC["s1"],
                                      imm2=RC["imm2"])
                s_prev = None
                for j in range(8):
                    h_t = hbp.tile([ROWS, X], F32, tag="h")
                    nc.scalar.activation(h_t[:], mag[:], AF.Tanh, scale=float(w1[j]))
                    s_new = hbp.tile([ROWS, X], F32, tag="s")
                    if s_prev is None:
                        nc.vector.tensor_scalar_mul(s_new[:], h_t[:], float(w2[j]))
                    else:
                        nc.vector.scalar_tensor_tensor(
                            s_new[:], h_t[:], float(w2[j]), s_prev[:],
                            OP.mult, OP.add)
                    s_prev = s_new
                p_t = midp.tile([ROWS, X], F32, tag="p")
                nc.vector.tensor_mul(p_t[:], s_prev[:], inv[:])
                xp_r = midp.tile([ROWS, X], F32, tag="xp_r")
                nc.vector.tensor_mul(xp_r[:], p_t[:], xr_t[:])
                xp_i = midp.tile([ROWS, X], F32, tag="xp_i")
                nc.vector.tensor_mul(xp_i[:], p_t[:], xi_t[:])
                if lw < X:
                    # pad region holds NaN (0/0); zero it so the Toeplitz
                    # band zeros actually mask it in the FIR matmuls
                    nc.vector.memset(xp_r[:, lw:X], 0.0)
                    nc.vector.memset(xp_i[:, lw:X], 0.0)

                stg = stgp.tile([ROWS, 2 * CW], F32, tag="stg")

                # windows, processed in pairs sharing PSUM banks
                for pj in range(0, nwin, 2):
                    npair = min(2, nwin - pj)
                    tb = tpsp.tile([128, 512], F32, tag="tps")
                    for u in range(npair):
                        j = pj + u
                        o = 256 * u
                        nc.tensor.transpose(
                            tb[:, o:o + 128], xp_r[:, STRIDE * j:STRIDE * j + 128],
                            eye_t[:])
                        nc.tensor.transpose(
                            tb[:, o + 128:o + 256],
                            xp_i[:, STRIDE * j:STRIDE * j + 128], eye_t[:])
                    xtb = xtbp.tile([128, 512], F32, tag="xtb")
                    if (pj // 2) % 2 == 0:
                        nc.vector.tensor_copy(xtb[:, 0:256 * npair], tb[:, 0:256 * npair])
                    else:
                        nc.scalar.copy(xtb[:, 0:256 * npair], tb[:, 0:256 * npair])

                    for u in range(npair):
                        j = pj + u
                        o = 256 * u
                        xrT = xtb[:, o:o + 128]
                        xiT = xtb[:, o + 128:o + 256]
                        ypr = yprp.tile([128, STRIDE], F32, tag="ypr")
                        ypi = ypip.tile([128, STRIDE], F32, tag="ypi")
                        nc.tensor.matmul(ypr[:], xrT, tr_t[:],
                                         start=True, stop=False)
                        nc.tensor.matmul(ypi[:], xrT, ti_t[:],
                                         start=True, stop=False)
                        nc.tensor.matmul(ypr[:], xiT, tni_t[:],
                                         start=False, stop=True)
                        nc.tensor.matmul(ypi[:], xiT, tr_t[:],
                                         start=False, stop=True)
                        # evacuate with scale + re/im interleave
                        so = 192 * j
                        if j % 2 == 0:
                            nc.scalar.mul(stg[:, so:so + 192:2], ypr[:], SCALE)
                            nc.vector.tensor_scalar_mul(
                                stg[:, so + 1:so + 192:2], ypi[:], SCALE)
                        else:
                            nc.vector.tensor_scalar_mul(
                                stg[:, so:so + 192:2], ypr[:], SCALE)
                            nc.scalar.mul(stg[:, so + 1:so + 192:2], ypi[:], SCALE)

                nc.sync.dma_start(out_d[:, 2 * cb:2 * (cb + vout)],
                                  stg[:, 0:2 * vout])


def _get_program(w1, w2, n_reps=1):
    key = (w1.tobytes(), w2.tobytes(), n_reps)
    if key not in _CACHE:
        _CACHE[key] = _build_program(w1, w2, n_reps)
    return _CACHE[key]


def _toeplitz(taps, sign=1.0):
    t = np.zeros((128, STRIDE), dtype=np.float32)
    for m in range(STRIDE):
        t[m:m + FL, m] = sign * taps
    return t


def kernel(x_real, x_imag, w_nl1, w_nl2, w_lin_real, w_lin_imag,
           _trace=False, _trace_kwargs=None):
    from concourse.bass_utils import run_bass_kernel_spmd

    w1 = np.asarray(w_nl1, dtype=np.float32).reshape(8)
    w2 = np.asarray(w_nl2, dtype=np.float32).reshape(8)
    wr = np.asarray(w_lin_real, dtype=np.float32).reshape(FL)
    wi = np.asarray(w_lin_imag, dtype=np.float32).reshape(FL)

    nc = _get_program(w1, w2)

    xr = np.ascontiguousarray(np.asarray(x_real, np.float32).reshape(B * H, W))
    xi = np.ascontiguousarray(np.asarray(x_imag, np.float32).reshape(B * H, W))
    consts = {
        "toe_r": _toeplitz(wr),
        "toe_i": _toeplitz(wi),
        "toe_ni": _toeplitz(wi, -1.0),
        "eye": np.eye(128, dtype=np.float32),
    }
    in_maps = []
    for c in range(NCORES):
        in_maps.append({
            "xr": np.ascontiguousarray(xr[ROWS * c:ROWS * (c + 1)]),
            "xi": np.ascontiguousarray(xi[ROWS * c:ROWS * (c + 1)]),
            **consts,
        })
    kw = {}
    if _trace:
        kw["trace"] = True
        if _trace_kwargs:
            kw.update(_trace_kwargs)
    res = run_bass_kernel_spmd(nc, in_maps, list(range(NCORES)), **kw)
    out = np.concatenate([res.results[c]["out"].reshape(ROWS, WOUT, 2)
                          for c in range(NCORES)], axis=0)
    out = out.reshape(B, H, WOUT, 1, 2)
    if _trace:
        kernel.last_results = res
    return out
